# revision 1
# baseline (speedup 1.0000x reference)
"""Trainium2 Bass kernel for nn_Attention_82051055223090.

ViT-style multi-head attention with RoPE on non-CLS tokens:
  qkv = x @ w_qkv + b_qkv ; rope(q,k) ; softmax(q k^T / sqrt(D)) v ; proj.

Strategy: pure data-parallel over batch (B=32 -> 4 per core x 8 cores), no
collectives.  Matmul operands are bf16 (full PE rate + fast weight load);
accumulation is fp32 in PSUM, softmax in fp32.  All layout transforms
(x transpose, rope-table transpose/swap, bias broadcast) happen host-side in
numpy during input marshalling so every device DMA is contiguous.

Per-core dataflow (per batch element):
  xT[c,t]   <- contiguous bf16 DMA (pre-transposed on host)   [768, 577]
  qkT       = w_qkv[:, :1536]^T @ x^T  (+b)                   [1536, 577]
  rope      in [d, t] layout; the rotate-half partition swap is done with
            four SBUF->SBUF DMAs (DVE ops cannot cross partitions)
  v         = x @ w_qkv[:, 1536:]  (+b), packed per head pair as
            [v_even | ones | v_odd]; the ones block makes the AV matmul
            also emit the softmax denominator broadcast over 64 rows
  scoresT   = kT-chunks(stationary) @ qT (moving)             [577, 577]
  expT      = exp(0.125 * scoresT)      (ACT, PSUM -> SBUF bf16)
  outT|den  = [v|ones]-as-lhsT @ expT   (PSUM fp32, accumulated over j)
  normOutT  = outT * reciprocal(den)    (reciprocal DMA-moved to out rows)
  out       = normOutT-as-lhsT @ w_proj + b_proj -> DMA out (fp32)
"""

import numpy as np

B, N, C, H, D = 32, 577, 768, 12, 64
NCORES = 8
NB = B // NCORES          # batches per core
P = 128
KT = C // P               # 6 contraction chunks of 128
NPAIR = H // 2            # 6 head pairs
TOK = [(i * P, min(P, N - i * P)) for i in range((N + P - 1) // P)]  # token tiles
NA = 289                  # i-chunk A = [0:289]
NBW = 288                 # i-chunk B = [289:577] (exact width)

_cache = {}


def _build():
    from contextlib import ExitStack

    import concourse.tile as tile
    from concourse import bacc, mybir
    from concourse.ap import AP

    f32 = mybir.dt.float32
    bf16 = mybir.dt.bfloat16
    AF = mybir.ActivationFunctionType
    OP = mybir.AluOpType

    nc = bacc.Bacc("TRN2", debug=False, enable_partition_id=False)

    xt_d = nc.dram_tensor("xt", [NB, C, N], bf16, kind="ExternalInput").ap()
    wqkv_d = nc.dram_tensor("w_qkv", [C, 3 * C], bf16, kind="ExternalInput").ap()
    wproj_d = nc.dram_tensor("w_proj", [C, C], bf16, kind="ExternalInput").ap()
    bqk_d = nc.dram_tensor("bqk2", [P, 18], f32, kind="ExternalInput").ap()
    bvb_d = nc.dram_tensor("bvb", [P, C], f32, kind="ExternalInput").ap()
    bpb_d = nc.dram_tensor("bpb", [P, C], f32, kind="ExternalInput").ap()
    cost_d = nc.dram_tensor("cost", [64, N], bf16, kind="ExternalInput").ap()
    sins_d = nc.dram_tensor("sins", [64, N], bf16, kind="ExternalInput").ap()
    out_d = nc.dram_tensor("out", [NB, N, C], f32, kind="ExternalOutput").ap()

    def ap3(base_ap, part_off, elem_off, dims):
        """Raw AP on the same tensor: partition slice + multi-dim free dims."""
        rowstr = base_ap.ap[0][0]
        return AP(
            base_ap.tensor,
            base_ap.offset + part_off * rowstr + elem_off,
            [[rowstr, dims[0]]] + [list(d) for d in dims[1:]],
        )

    with tile.TileContext(nc) as tc, ExitStack() as ctx:
        const = ctx.enter_context(tc.tile_pool(name="const", bufs=1))
        ps = ctx.enter_context(tc.tile_pool(name="ps", bufs=4, space="PSUM"))
        scp = ctx.enter_context(tc.tile_pool(name="scp", bufs=2, space="PSUM"))
        sb = ctx.enter_context(tc.tile_pool(name="sb", bufs=1))

        # ---- constants (all pre-formatted on host, contiguous DMAs) ----
        w_sb = []
        for k in range(KT):
            w = const.tile([P, 3 * C], bf16, tag=f"w{k}", name=f"w{k}")
            nc.sync.dma_start(w, wqkv_d[k * P:(k + 1) * P, :])
            w_sb.append(w)
        wp_sb = []
        for k in range(KT):
            wp = const.tile([P, C], bf16, tag=f"wp{k}", name=f"wp{k}")
            nc.sync.dma_start(wp, wproj_d[k * P:(k + 1) * P, :])
            wp_sb.append(wp)

        cosT = const.tile([P, N], bf16, tag="cosT", name="cosT")
        sinS = const.tile([P, N], bf16, tag="sinS", name="sinS")
        for g in range(2):  # duplicate across the two 64-partition groups
            nc.sync.dma_start(cosT[g * 64:g * 64 + 64, :], cost_d)
            nc.sync.dma_start(sinS[g * 64:g * 64 + 64, :], sins_d)

        bqk = const.tile([P, 18], f32, tag="bqk", name="bqk")
        nc.sync.dma_start(bqk, bqk_d)
        bvB = const.tile([P, C], f32, tag="bvB", name="bvB")
        nc.sync.dma_start(bvB, bvb_d)
        bpB = const.tile([P, C], f32, tag="bpB", name="bpB")
        nc.sync.dma_start(bpB, bpb_d)

        # Deferred emission: attention of pair p is emitted after qkT+rope of
        # the NEXT pair, so the rope chain latency (DVE + swap DMAs) hides
        # behind the next pair's PE work; proj(b) is emitted right after the
        # last attention of batch b (inside batch b+1's stream).

        def emit_front(b):
            xts = []
            for k in range(KT):
                xt = sb.tile([P, N], bf16, tag="xt", bufs=12, name=f"xt{b}_{k}")
                nc.sync.dma_start(xt, xt_d[b, k * P:(k + 1) * P, :])
                xts.append(xt)
            vts = []
            for it, (ts, tsz) in enumerate(TOK):
                vt = sb.tile([P, NPAIR * 192], bf16, tag="v", bufs=11, name=f"v{b}_{it}")
                for half in range(2):
                    pv = ps.tile([P, 512], f32, tag="ps", name=f"pv{b}_{it}_{half}")
                    c0 = 2 * C + half * 384
                    for k in range(KT):
                        nc.tensor.matmul(
                            pv[0:tsz, 0:384],
                            xts[k][:, ts:ts + tsz],
                            w_sb[k][:, c0:c0 + 384],
                            start=(k == 0), stop=(k == KT - 1))
                    po = 0 if half == 0 else 576
                    dst = ap3(vt[:], 0, po, [tsz, (192, 3), (128, 2), (1, 64)])
                    src_ = pv[0:tsz, 0:384].rearrange("p (a c d) -> p a c d", a=3, c=2)
                    bsrc = bvB[0:tsz, half * 384:(half + 1) * 384].rearrange(
                        "p (a c d) -> p a c d", a=3, c=2)
                    nc.vector.tensor_tensor(dst, src_, bsrc, OP.add)
                ones = ap3(vt[:], 0, 64, [tsz, (192, NPAIR), (1, 64)])
                nc.vector.memset(ones, 1.0)
                vts.append(vt)
            return xts, vts

        def emit_qk(b, pair, xts):
            qf_pair = []
            for m in (pair, 6 + pair):   # q tile then k tile
                pA = ps.tile([P, 512], f32, tag="ps", name=f"pqk{b}_{m}_a")
                pB = ps.tile([P, 512], f32, tag="ps", name=f"pqk{b}_{m}_b")
                for k in range(KT):
                    nc.tensor.matmul(
                        pA[:, 0:NA], w_sb[k][:, m * P:(m + 1) * P],
                        xts[k][:, 0:NA],
                        start=(k == 0), stop=(k == KT - 1))
                for k in range(KT):
                    nc.tensor.matmul(
                        pB[:, 0:NBW], w_sb[k][:, m * P:(m + 1) * P],
                        xts[k][:, NA:N],
                        start=(k == 0), stop=(k == KT - 1))
                qb = sb.tile([P, N], bf16, tag="qb", bufs=6, name=f"qb{b}_{m}")
                nc.vector.tensor_scalar(
                    qb[:, 0:NA], pA[:, 0:NA], bqk[:, m:m + 1], None, OP.add)
                nc.vector.tensor_scalar(
                    qb[:, NA:N], pB[:, 0:NBW], bqk[:, m:m + 1], None, OP.add)
                # rope: qf = qb*cosT + swap32(qb*sinS); the 32-block swap is
                # four SBUF->SBUF DMAs (partition moves are illegal on DVE)
                qf = sb.tile([P, N], bf16, tag="qf", bufs=12, name=f"qf{b}_{m}")
                ut = sb.tile([P, N], bf16, tag="ut", bufs=4, name=f"ut{b}_{m}")
                us = sb.tile([P, N], bf16, tag="us", bufs=4, name=f"us{b}_{m}")
                nc.vector.tensor_tensor(qf[:], qb[:], cosT[:], OP.mult)
                nc.vector.tensor_tensor(ut[:], qb[:], sinS[:], OP.mult)
                for blk in range(4):
                    o0, i0 = blk * 32, (blk ^ 1) * 32
                    nc.sync.dma_start(us[o0:o0 + 32, :], ut[i0:i0 + 32, :])
                nc.vector.tensor_tensor(qf[:], qf[:], us[:], OP.add)
                qf_pair.append(qf)
            return qf_pair

        def emit_attention(b, pair, qft, kft, vts, no_sb):
            no_t = sb.tile([P, N], bf16, tag="no", bufs=9, name=f"no{b}_{pair}")
            for half in range(2):
                h0 = half * 64
                drow = 64 - h0   # denom rows (opposite 64-block)
                avA = ps.tile([P, 512], f32, tag="ps", name=f"avA{b}_{pair}_{half}")
                avB = ps.tile([P, 512], f32, tag="ps", name=f"avB{b}_{pair}_{half}")
                for jc, (js, jsz) in enumerate(TOK):
                    sct = scp.tile([P, 1024], f32, tag="sc", name=f"sc{b}_{pair}_{half}_{jc}")
                    nc.tensor.matmul(
                        sct[0:jsz, 0:NA], kft[h0:h0 + 64, js:js + jsz],
                        qft[h0:h0 + 64, 0:NA], skip_group_check=True)
                    nc.tensor.matmul(
                        sct[0:jsz, 512:512 + NA], kft[h0:h0 + 64, js:js + jsz],
                        qft[h0:h0 + 64, NA - 1:N], skip_group_check=True)
                    et = sb.tile([P, 2 * NA], bf16, tag="e", bufs=8, name=f"e{b}_{pair}_{half}_{jc}")
                    nc.scalar.activation(
                        et[0:jsz].rearrange("p (a q) -> p a q", a=2),
                        sct[0:jsz].rearrange("p (a q) -> p a q", a=2)[:, :, 0:NA],
                        AF.Exp, scale=0.125)
                    vslice = vts[jc][0:jsz, pair * 192 + h0:pair * 192 + h0 + 128]
                    nc.tensor.matmul(
                        avA[:, 0:NA], vslice, et[0:jsz, 0:NA],
                        start=(jc == 0), stop=(jc == 4), skip_group_check=True)
                    nc.tensor.matmul(
                        avB[:, 0:NBW], vslice, et[0:jsz, NA + 1:2 * NA],
                        start=(jc == 0), stop=(jc == 4), skip_group_check=True)
                # pipelined normalize: each chunk's reciprocal -> partition-move
                # DMA -> multiply chain runs independently so the PSUM slots
                # free ~1us earlier per head
                rec = sb.tile([P, N], f32, tag="rec", bufs=4, name=f"rec{b}_{pair}_{half}")
                nc.vector.reciprocal(rec[drow:drow + 64, 0:NA], avA[drow:drow + 64, 0:NA])
                nc.sync.dma_start(rec[h0:h0 + 64, 0:NA], rec[drow:drow + 64, 0:NA])
                nc.vector.reciprocal(rec[drow:drow + 64, NA:N], avB[drow:drow + 64, 0:NBW])
                nc.sync.dma_start(rec[h0:h0 + 64, NA:N], rec[drow:drow + 64, NA:N])
                nc.vector.tensor_tensor(
                    no_t[h0:h0 + 64, 0:NA], avA[h0:h0 + 64, 0:NA],
                    rec[h0:h0 + 64, 0:NA], OP.mult)
                nc.vector.tensor_tensor(
                    no_t[h0:h0 + 64, NA:N], avB[h0:h0 + 64, 0:NBW],
                    rec[h0:h0 + 64, NA:N], OP.mult)
            no_sb.append(no_t)

        def emit_proj(b, no_sb):
            for it, (ts, tsz) in enumerate(TOK):
                ot = sb.tile([P, C], f32, tag="outp", bufs=4, name=f"o{b}_{it}")
                for half in range(2):
                    pp = ps.tile([P, 512], f32, tag="ps", name=f"pp{b}_{it}_{half}")
                    c0 = half * 384
                    for kk in range(KT):
                        nc.tensor.matmul(
                            pp[0:tsz, 0:384], no_sb[kk][:, ts:ts + tsz],
                            wp_sb[kk][:, c0:c0 + 384],
                            start=(kk == 0), stop=(kk == KT - 1))
                    nc.vector.tensor_tensor(
                        ot[0:tsz, c0:c0 + 384], pp[0:tsz, 0:384],
                        bpB[0:tsz, c0:c0 + 384], OP.add)
                nc.sync.dma_start(out_d[b, ts:ts + tsz, :], ot[0:tsz, :])

        pending = []
        state = {}

        def pop_attn():
            pb, pp_, pq, pk = pending.pop(0)
            emit_attention(pb, pp_, pq, pk, state[pb]["vts"], state[pb]["no_sb"])
            if pp_ == NPAIR - 1:
                emit_proj(pb, state[pb]["no_sb"])

        for b in range(NB):
            xts, vts = emit_front(b)
            state[b] = dict(vts=vts, no_sb=[])
            for pair in range(NPAIR):
                qf_pair = emit_qk(b, pair, xts)
                if len(pending) >= 2:
                    pop_attn()
                pending.append((b, pair, qf_pair[0], qf_pair[1]))
        while pending:
            pop_attn()


    nc.compile()
    return nc


def _get_nc():
    if "nc" not in _cache:
        _cache["nc"] = _build()
    return _cache["nc"]


def _prep_shared(inputs):
    """Host-side layout prep shared across cores (numpy only)."""
    import ml_dtypes

    bf = ml_dtypes.bfloat16
    w_qkv = np.ascontiguousarray(np.asarray(inputs["w_qkv"], np.float32)).astype(bf)
    w_proj = np.ascontiguousarray(np.asarray(inputs["w_proj"], np.float32)).astype(bf)
    b_qkv = np.asarray(inputs["b_qkv"], np.float32)
    b_proj = np.asarray(inputs["b_proj"], np.float32)
    sin = np.asarray(inputs["rope_sin"], np.float32)  # [576, 64]
    cos = np.asarray(inputs["rope_cos"], np.float32)

    bqk2 = np.ascontiguousarray(b_qkv.reshape(18, P).T)          # [128, 18]
    bvb = np.ascontiguousarray(np.broadcast_to(b_qkv[2 * C:], (P, C)))
    bpb = np.ascontiguousarray(np.broadcast_to(b_proj, (P, C)))

    cost = np.ones((64, N), np.float32)
    cost[:, 1:] = cos.T
    # sinS holds sin at the swapped index with the rotate-half sign pattern:
    # rows 0:32 <- +sin cols 32:64 ; rows 32:64 <- -sin cols 0:32
    sins = np.zeros((64, N), np.float32)
    sins[0:32, 1:] = sin.T[32:64]
    sins[32:64, 1:] = -sin.T[0:32]

    return {
        "w_qkv": w_qkv,
        "w_proj": w_proj,
        "bqk2": bqk2.astype(np.float32),
        "bvb": bvb.astype(np.float32),
        "bpb": bpb.astype(np.float32),
        "cost": cost.astype(bf),
        "sins": sins.astype(bf),
    }


last_results = None


def kernel(**inputs):
    global last_results
    import ml_dtypes

    from concourse.bass_utils import run_bass_kernel_spmd

    nc = _get_nc()
    bf = ml_dtypes.bfloat16
    x = np.asarray(inputs["x"], np.float32)
    # host-side transpose + bf16 cast: [B, N, C] -> [B, C, N]
    xt_all = np.ascontiguousarray(x.transpose(0, 2, 1)).astype(bf)
    shared = _prep_shared(inputs)

    in_maps = []
    for c in range(NCORES):
        m = dict(shared)
        m["xt"] = np.ascontiguousarray(xt_all[c * NB:(c + 1) * NB])
        in_maps.append(m)

    res = run_bass_kernel_spmd(nc, in_maps, core_ids=list(range(NCORES)))
    last_results = res
    return np.concatenate([res.results[c]["out"] for c in range(NCORES)], axis=0)

